# revision 6
# baseline (speedup 1.0000x reference)
"""Class-balanced segmentation loss on 8 Trainium2 NeuronCores.

Math: with counts_c = #{p: t_p == c}, S_c = sum_{p: t_p=c} logsumexp_p,
T_c = sum_{p: t_p=c} pred[c, p], and w_c = 0.001 / (1 - 0.999**counts_c)
(0 for empty classes), the reference loss is

    loss = sum_c w_c * (S_c - T_c) / sum_c w_c * counts_c .

Everything is linear in w, so the device does a single data-parallel pass
(one batch per core) producing per-core partials (counts, S, T) and the
19-float weight/loss arithmetic runs on the host after the gather.
"""

import numpy as np

NCLASS = 19
B, H, W = 8, 512, 512
NPIX = H * W          # 262144 pixels per batch
P = 128               # SBUF partitions
FW = NPIX // P        # 2048 free-dim elements per partition
NCORES = 8

# tile/chunk config
F = 512               # free-dim chunk size
NCH = FW // F         # chunks per batch

_COMPILED = {}


def _patch_tile_drain():
    """walrus in this container rejects >1-2 sem-waits on one instruction
    ("Too many sync wait commands"); the tile-exit Drain carries one wait
    per logical processor. Split them into single-wait NOPs."""
    import bass_rust
    import concourse.tile as tile

    if getattr(tile.TileContext, "_drain_patched", False):
        return

    def _drain_and_barrier(self, tick_clock, wait_clock):
        from concourse.tile import ScopedClock

        probe = self.nc.sync.nop(nofuse=True)
        wait_clock.add_sem_waits(
            probe.ins, ScopedClock({None: tick_clock.global_clock})
        )
        si = probe.ins.sync_info
        waits = list(si.on_wait) if si else []
        if si:
            si.on_wait = waits[:1]
        for i in range(1, len(waits)):
            n = self.nc.sync.nop(nofuse=True)
            n.ins.sync_info = bass_rust.SyncInfo(
                on_wait=waits[i : i + 1], on_update=[]
            )
        self.nc.sync.drain()
        self.nc.all_engine_barrier()
        assert self.sems is not None
        popped = self.nc._tile_sem_poison_stack.pop()
        assert popped is self._sem_poison
        self.nc.clear_and_free_semaphores(list(self.sems.allocated().values()))
        self.nc.all_engine_barrier()

    tile.TileContext._drain_and_barrier = _drain_and_barrier
    tile.TileContext._drain_patched = True


def _split_excess_waits(nc, maxw=1):
    """Post-pass: any instruction carrying more than `maxw` sem-waits gets
    the extras moved onto same-engine NOPs inserted right before it (the
    engine executes in order, so semantics are identical)."""
    import bass_rust
    from concourse import mybir

    for blk in nc.m.functions[0].blocks:
        insts = list(blk.instructions)
        out = []
        changed = False
        for inst in insts:
            si = inst.sync_info
            if si is not None and si.on_wait and len(si.on_wait) > maxw:
                waits = list(si.on_wait)
                si.on_wait = waits[:maxw]
                extra = waits[maxw:]
                eng = nc.engines[inst.engine]
                for i in range(0, len(extra), maxw):
                    n = eng.nop(nofuse=True)
                    # the nop was appended to the current bb; move it here
                    cur = nc.cur_bb.bb
                    cur_insts = list(cur.instructions)
                    assert cur_insts[-1].name == n.ins.name
                    cur.instructions = cur_insts[:-1]
                    n.ins.sync_info = bass_rust.SyncInfo(
                        on_wait=extra[i : i + maxw], on_update=[]
                    )
                    out.append(n.ins)
                changed = True
            out.append(inst)
        if changed:
            blk.instructions = out


def build_nc(reps: int = 1):
    """Build the per-core Bass program (SPMD: every core runs this on its
    own batch). reps>1 repeats the compute for wall-clock HW timing.

    Host passes pred pre-transposed to [P, NCH, NCLASS, F] so each chunk
    is one contiguous-per-partition DMA (38 KB runs)."""
    from contextlib import ExitStack

    import concourse.bass as bass
    import concourse.tile as tile
    from concourse import mybir

    _patch_tile_drain()

    nc = bass.Bass()
    pred = nc.declare_dram_parameter(
        "pred", [P, NCH, NCLASS, F], mybir.dt.float32, isOutput=False
    )
    targ = nc.declare_dram_parameter(
        "targ", [P, FW], mybir.dt.float32, isOutput=False
    )
    # columns: [0:NCH*19] = S, [NCH*19:2*NCH*19] = T, [2*NCH*19:3*NCH*19] = counts
    out = nc.declare_dram_parameter(
        "out", [P, 3 * NCH * NCLASS], mybir.dt.float32, isOutput=True
    )

    with tile.TileContext(nc) as tc:
        with ExitStack() as ctx:
            io = ctx.enter_context(tc.tile_pool(name="io", bufs=2))
            work = ctx.enter_context(tc.tile_pool(name="work", bufs=2))
            acc = ctx.enter_context(tc.tile_pool(name="acc", bufs=1))

            s_acc = acc.tile([P, NCH * NCLASS], mybir.dt.float32)
            t_acc = acc.tile([P, NCH * NCLASS], mybir.dt.float32)
            c_acc = acc.tile([P, NCH * NCLASS], mybir.dt.float32)

            for _ in range(reps):
                for k in range(NCH):
                    p_tile = io.tile([P, NCLASS, F], mybir.dt.float32, tag="p")
                    nc.sync.dma_start(out=p_tile[:, :, :], in_=pred[:, k, :, :])
                    t_tile = io.tile([P, F], mybir.dt.float32, tag="t")
                    nc.sync.dma_start(
                        out=t_tile[:], in_=targ[:, k * F : (k + 1) * F]
                    )

                    # exp of all classes in one ACT instruction
                    e_tile = work.tile([P, NCLASS, F], mybir.dt.float32, tag="e")
                    nc.scalar.activation(
                        out=e_tile[:, :, :],
                        in_=p_tile[:, :, :],
                        func=mybir.ActivationFunctionType.Exp,
                    )
                    # sum over classes (innermost axis after permute)
                    sx = work.tile([P, F], mybir.dt.float32, tag="sx")
                    nc.vector.tensor_reduce(
                        out=sx[:],
                        in_=e_tile[:, :, :].rearrange("p c f -> p f c"),
                        axis=mybir.AxisListType.X,
                        op=mybir.AluOpType.add,
                    )
                    lse = work.tile([P, F], mybir.dt.float32, tag="lse")
                    nc.scalar.activation(
                        out=lse[:],
                        in_=sx[:],
                        func=mybir.ActivationFunctionType.Ln,
                    )

                    scr = work.tile([P, F], mybir.dt.float32, tag="scr")
                    for c in range(NCLASS):
                        col = k * NCLASS + c
                        # S_c partial: sum_f (t==c) * lse
                        nc.vector.scalar_tensor_tensor(
                            out=scr[:],
                            in0=t_tile[:],
                            scalar=float(c),
                            in1=lse[:],
                            op0=mybir.AluOpType.is_equal,
                            op1=mybir.AluOpType.mult,
                            accum_out=s_acc[:, col : col + 1],
                        )
                        # T_c partial: sum_f (t==c) * pred_c
                        nc.vector.scalar_tensor_tensor(
                            out=scr[:],
                            in0=t_tile[:],
                            scalar=float(c),
                            in1=p_tile[:, c, :],
                            op0=mybir.AluOpType.is_equal,
                            op1=mybir.AluOpType.mult,
                            accum_out=t_acc[:, col : col + 1],
                        )
                        # counts partial: accum = add-reduce of (t==c), +0 seed
                        nc.vector.tensor_scalar(
                            out=scr[:],
                            in0=t_tile[:],
                            scalar1=float(c),
                            scalar2=0.0,
                            op0=mybir.AluOpType.is_equal,
                            op1=mybir.AluOpType.add,
                            accum_out=c_acc[:, col : col + 1],
                        )

            nco = NCH * NCLASS
            nc.sync.dma_start(out=out[:, 0 * nco : 1 * nco], in_=s_acc[:])
            nc.sync.dma_start(out=out[:, 1 * nco : 2 * nco], in_=t_acc[:])
            nc.sync.dma_start(out=out[:, 2 * nco : 3 * nco], in_=c_acc[:])

    _split_excess_waits(nc, maxw=1)
    return nc


def _shard_inputs(pred_np, targ_np):
    in_maps = []
    for b in range(NCORES):
        # [19, 262144] -> [P, NCH, NCLASS, F]
        pb = (
            pred_np[b]
            .reshape(NCLASS, P, NCH, F)
            .transpose(1, 2, 0, 3)
        )
        in_maps.append(
            {
                "pred": np.ascontiguousarray(pb, dtype=np.float32),
                "targ": targ_np[b].reshape(P, FW).astype(np.float32),
            }
        )
    return in_maps


def _run_device(pred_np, targ_np, reps: int = 1, in_maps=None):
    """Shard batch-wise over the 8 cores, run the SPMD program, return the
    per-core [P, 3*NCH*19] partial tensors."""
    from concourse.bass_utils import run_bass_kernel_spmd

    if reps not in _COMPILED:
        _COMPILED[reps] = build_nc(reps)
    nc = _COMPILED[reps]

    if in_maps is None:
        in_maps = _shard_inputs(pred_np, targ_np)
    res = run_bass_kernel_spmd(nc, in_maps, core_ids=list(range(NCORES)))
    return [res.results[i]["out"] for i in range(NCORES)]


def _finish(outs):
    """Host epilogue: gather/all-reduce the 3x19 partials and apply the
    class-balanced weight formula (matches reference semantics)."""
    nco = NCH * NCLASS
    S = np.zeros(NCLASS, np.float64)
    T = np.zeros(NCLASS, np.float64)
    C = np.zeros(NCLASS, np.float64)
    for o in outs:
        o = np.asarray(o, np.float64)
        S += o[:, 0 * nco : 1 * nco].reshape(P, NCH, NCLASS).sum((0, 1))
        T += o[:, 1 * nco : 2 * nco].reshape(P, NCH, NCLASS).sum((0, 1))
        C += o[:, 2 * nco : 3 * nco].reshape(P, NCH, NCLASS).sum((0, 1))
    beta = 1.0 - 0.001
    with np.errstate(divide="ignore", over="ignore", under="ignore"):
        w = (1.0 - beta) / (1.0 - beta**C)
    w = np.where(C > 0, w, 0.0)
    num = float(np.sum(w * (S - T)))
    den = float(np.sum(w * C))
    return np.array(np.float32(num / den))


def kernel(pred: np.ndarray, target: np.ndarray) -> np.ndarray:
    pred_np = np.asarray(pred, dtype=np.float32)
    targ_np = np.asarray(target)
    outs = _run_device(pred_np, targ_np, reps=1)
    return _finish(outs)


# revision 8
# speedup vs baseline: 43.7210x; 43.7210x over previous
"""Class-balanced segmentation loss on 8 Trainium2 NeuronCores.

Math: with counts_c = #{p: t_p == c}, S_c = sum_{p: t_p=c} logsumexp_p,
T_c = sum_{p: t_p=c} pred[c, p], and w_c = 0.001 / (1 - 0.999**counts_c)
(0 for empty classes), the reference loss is

    loss = sum_c w_c * (S_c - T_c) / sum_c w_c * counts_c .

Everything is linear in w, so the device does a single data-parallel pass
(one batch per core) producing per-core partials (counts, S, T) and the
19-float weight/loss arithmetic runs on the host after the gather.
"""

import numpy as np

NCLASS = 19
B, H, W = 8, 512, 512
NPIX = H * W          # 262144 pixels per batch
P = 128               # SBUF partitions
FW = NPIX // P        # 2048 free-dim elements per partition
NCORES = 8

# tile/chunk config
F = 512               # free-dim chunk size
NCH = FW // F         # chunks per batch

_COMPILED = {}


def _patch_tile_drain():
    """walrus in this container rejects >1-2 sem-waits on one instruction
    ("Too many sync wait commands"); the tile-exit Drain carries one wait
    per logical processor. Split them into single-wait NOPs."""
    import bass_rust
    import concourse.tile as tile

    if getattr(tile.TileContext, "_drain_patched", False):
        return

    def _drain_and_barrier(self, tick_clock, wait_clock):
        from concourse.tile import ScopedClock

        probe = self.nc.sync.nop(nofuse=True)
        wait_clock.add_sem_waits(
            probe.ins, ScopedClock({None: tick_clock.global_clock})
        )
        si = probe.ins.sync_info
        waits = list(si.on_wait) if si else []
        if si:
            si.on_wait = waits[:1]
        for i in range(1, len(waits)):
            n = self.nc.sync.nop(nofuse=True)
            n.ins.sync_info = bass_rust.SyncInfo(
                on_wait=waits[i : i + 1], on_update=[]
            )
        self.nc.sync.drain()
        self.nc.all_engine_barrier()
        assert self.sems is not None
        popped = self.nc._tile_sem_poison_stack.pop()
        assert popped is self._sem_poison
        self.nc.clear_and_free_semaphores(list(self.sems.allocated().values()))
        self.nc.all_engine_barrier()

    tile.TileContext._drain_and_barrier = _drain_and_barrier
    tile.TileContext._drain_patched = True


def _split_excess_waits(nc, maxw=1):
    """Post-pass: any instruction carrying more than `maxw` sem-waits gets
    the extras moved onto same-engine NOPs inserted right before it (the
    engine executes in order, so semantics are identical)."""
    import bass_rust
    from concourse import mybir

    for blk in nc.m.functions[0].blocks:
        insts = list(blk.instructions)
        out = []
        changed = False
        for inst in insts:
            si = inst.sync_info
            if si is not None and si.on_wait and len(si.on_wait) > maxw:
                waits = list(si.on_wait)
                si.on_wait = waits[:maxw]
                extra = waits[maxw:]
                eng = nc.engines[inst.engine]
                for i in range(0, len(extra), maxw):
                    n = eng.nop(nofuse=True)
                    # the nop was appended to the current bb; move it here
                    cur = nc.cur_bb.bb
                    cur_insts = list(cur.instructions)
                    assert cur_insts[-1].name == n.ins.name
                    cur.instructions = cur_insts[:-1]
                    n.ins.sync_info = bass_rust.SyncInfo(
                        on_wait=extra[i : i + maxw], on_update=[]
                    )
                    out.append(n.ins)
                changed = True
            out.append(inst)
        if changed:
            blk.instructions = out


def build_nc(reps: int = 1):
    """Build the per-core Bass program (SPMD: every core runs this on its
    own batch). reps>1 repeats the compute for wall-clock HW timing.

    Host passes pred pre-transposed to [P, NCH, NCLASS, F] so each chunk
    is one contiguous-per-partition DMA (38 KB runs)."""
    from contextlib import ExitStack

    import concourse.bass as bass
    import concourse.tile as tile
    from concourse import mybir

    _patch_tile_drain()

    nc = bass.Bass()
    pred = nc.declare_dram_parameter(
        "pred", [P, NCH, NCLASS, F], mybir.dt.float32, isOutput=False
    )
    targ = nc.declare_dram_parameter(
        "targ", [P, FW], mybir.dt.float32, isOutput=False
    )
    # columns: [0:NCH*19] = S, [NCH*19:2*NCH*19] = T, [2*NCH*19:3*NCH*19] = counts
    out = nc.declare_dram_parameter(
        "out", [P, 3 * NCH * NCLASS], mybir.dt.float32, isOutput=True
    )

    with tile.TileContext(nc) as tc:
        with ExitStack() as ctx:
            io = ctx.enter_context(tc.tile_pool(name="io", bufs=2))
            work = ctx.enter_context(tc.tile_pool(name="work", bufs=2))
            acc = ctx.enter_context(tc.tile_pool(name="acc", bufs=1))

            s_acc = acc.tile([P, NCH * NCLASS], mybir.dt.float32)
            t_acc = acc.tile([P, NCH * NCLASS], mybir.dt.float32)
            c_acc = acc.tile([P, NCH * NCLASS], mybir.dt.float32)

            def _body():
                for k in range(NCH):
                    p_tile = io.tile([P, NCLASS, F], mybir.dt.float32, tag="p")
                    nc.sync.dma_start(out=p_tile[:, :, :], in_=pred[:, k, :, :])
                    t_tile = io.tile([P, F], mybir.dt.float32, tag="t")
                    nc.sync.dma_start(
                        out=t_tile[:], in_=targ[:, k * F : (k + 1) * F]
                    )

                    # exp of all classes in one ACT instruction
                    e_tile = work.tile([P, NCLASS, F], mybir.dt.float32, tag="e")
                    nc.scalar.activation(
                        out=e_tile[:, :, :],
                        in_=p_tile[:, :, :],
                        func=mybir.ActivationFunctionType.Exp,
                    )
                    # sum over classes (innermost axis after permute)
                    sx = work.tile([P, F], mybir.dt.float32, tag="sx")
                    nc.vector.tensor_reduce(
                        out=sx[:],
                        in_=e_tile[:, :, :].rearrange("p c f -> p f c"),
                        axis=mybir.AxisListType.X,
                        op=mybir.AluOpType.add,
                    )
                    lse = work.tile([P, F], mybir.dt.float32, tag="lse")
                    nc.scalar.activation(
                        out=lse[:],
                        in_=sx[:],
                        func=mybir.ActivationFunctionType.Ln,
                    )

                    scr = work.tile([P, F], mybir.dt.float32, tag="scr")
                    for c in range(NCLASS):
                        col = k * NCLASS + c
                        # S_c partial: sum_f (t==c) * lse
                        nc.vector.scalar_tensor_tensor(
                            out=scr[:],
                            in0=t_tile[:],
                            scalar=float(c),
                            in1=lse[:],
                            op0=mybir.AluOpType.is_equal,
                            op1=mybir.AluOpType.mult,
                            accum_out=s_acc[:, col : col + 1],
                        )
                        # T_c partial: sum_f (t==c) * pred_c
                        nc.vector.scalar_tensor_tensor(
                            out=scr[:],
                            in0=t_tile[:],
                            scalar=float(c),
                            in1=p_tile[:, c, :],
                            op0=mybir.AluOpType.is_equal,
                            op1=mybir.AluOpType.mult,
                            accum_out=t_acc[:, col : col + 1],
                        )
                        # counts partial: accum = add-reduce of (t==c), +0 seed
                        nc.vector.tensor_scalar(
                            out=scr[:],
                            in0=t_tile[:],
                            scalar1=float(c),
                            scalar2=0.0,
                            op0=mybir.AluOpType.is_equal,
                            op1=mybir.AluOpType.add,
                            accum_out=c_acc[:, col : col + 1],
                        )

            if reps == 1:
                _body()
            else:
                with tc.For_i(0, reps, 1):
                    _body()

            nco = NCH * NCLASS
            nc.sync.dma_start(out=out[:, 0 * nco : 1 * nco], in_=s_acc[:])
            nc.sync.dma_start(out=out[:, 1 * nco : 2 * nco], in_=t_acc[:])
            nc.sync.dma_start(out=out[:, 2 * nco : 3 * nco], in_=c_acc[:])

    _split_excess_waits(nc, maxw=1)
    return nc


def _shard_inputs(pred_np, targ_np):
    in_maps = []
    for b in range(NCORES):
        # [19, 262144] -> [P, NCH, NCLASS, F]
        pb = (
            pred_np[b]
            .reshape(NCLASS, P, NCH, F)
            .transpose(1, 2, 0, 3)
        )
        in_maps.append(
            {
                "pred": np.ascontiguousarray(pb, dtype=np.float32),
                "targ": targ_np[b].reshape(P, FW).astype(np.float32),
            }
        )
    return in_maps


def _run_device(pred_np, targ_np, reps: int = 1, in_maps=None):
    """Shard batch-wise over the 8 cores, run the SPMD program, return the
    per-core [P, 3*NCH*19] partial tensors."""
    from concourse.bass_utils import run_bass_kernel_spmd

    if reps not in _COMPILED:
        _COMPILED[reps] = build_nc(reps)
    nc = _COMPILED[reps]

    if in_maps is None:
        in_maps = _shard_inputs(pred_np, targ_np)
    res = run_bass_kernel_spmd(nc, in_maps, core_ids=list(range(NCORES)))
    return [res.results[i]["out"] for i in range(NCORES)]


def _finish(outs):
    """Host epilogue: gather/all-reduce the 3x19 partials and apply the
    class-balanced weight formula (matches reference semantics)."""
    nco = NCH * NCLASS
    S = np.zeros(NCLASS, np.float64)
    T = np.zeros(NCLASS, np.float64)
    C = np.zeros(NCLASS, np.float64)
    for o in outs:
        o = np.asarray(o, np.float64)
        S += o[:, 0 * nco : 1 * nco].reshape(P, NCH, NCLASS).sum((0, 1))
        T += o[:, 1 * nco : 2 * nco].reshape(P, NCH, NCLASS).sum((0, 1))
        C += o[:, 2 * nco : 3 * nco].reshape(P, NCH, NCLASS).sum((0, 1))
    beta = 1.0 - 0.001
    with np.errstate(divide="ignore", over="ignore", under="ignore"):
        w = (1.0 - beta) / (1.0 - beta**C)
    w = np.where(C > 0, w, 0.0)
    num = float(np.sum(w * (S - T)))
    den = float(np.sum(w * C))
    return np.array(np.float32(num / den))


def kernel(pred: np.ndarray, target: np.ndarray) -> np.ndarray:
    pred_np = np.asarray(pred, dtype=np.float32)
    targ_np = np.asarray(target)
    outs = _run_device(pred_np, targ_np, reps=1)
    return _finish(outs)
